# revision 1
# baseline (speedup 1.0000x reference)
"""Distributed CBoE (single-head attention over an embedding table) for 8 trn2 cores.

out = softmax(x @ E^T) @ E,  x:[4096,1024] f32, E:[32768,1024] f32.

Strategy: shard E along N (4096 rows/core). Each core computes, for all 4096
tokens, a flash-style partial softmax over its shard:
  m_c = rowmax(x @ E_c^T), l_c = rowsum(exp(s - m_c)), o_c = exp(s - m_c) @ E_c / l_c
The host combines shards: out = sum_c w_c * o_c, w_c = l_c e^{m_c - M} / sum(...).

Per-core kernel (token chunks of 256 = 2 subtiles of 128):
  pass A: scores chunk [256, 4096] = xT_chunk.T @ ET_shard (f32r matmuls,
          lhsT = xT tiles stationary, rhs = resident ET in SBUF), psum -> SBUF
          scores cache (fp32) + running row-max on DVE.
  pass B: P = exp(scores - m) on ACT (bf16 out, accum_out gives row-sums),
          PE-transpose P tiles -> P^T (bf16), mm2: acc[t,d] += P^T.T @ E_nat
          (bf16, E_nat streamed from DRAM), normalize by 1/l, DMA out.

Layout tricks: host passes x^T, E^T (so no on-chip transposes of inputs) and
E as bf16 (halves mm2 stream traffic).
"""

import sys

if "/opt/trn_rl_repo" not in sys.path:
    sys.path.insert(0, "/opt/trn_rl_repo")

import numpy as np
import ml_dtypes

import concourse.bass as bass
import concourse.mybir as mybir
import concourse.tile as tile
from concourse import bacc
from concourse.bass_utils import run_bass_kernel_spmd
from concourse.masks import make_identity

F32 = mybir.dt.float32
F32R = mybir.dt.float32r
BF16 = mybir.dt.bfloat16
AX = mybir.AxisListType.X
EXP = mybir.ActivationFunctionType.Exp

T, N, D = 4096, 32768, 1024
NCORES = 8
NSH = N // NCORES  # 4096 embedding rows per core


def build_nc(t=T, d=D, nsh=NSH, tc_tokens=256, do_compile=True):
    """Build the per-core Bass program (SPMD; all cores run the same NEFF)."""
    KC = d // 128          # contraction chunks for mm1
    TSUB = tc_tokens // 128  # token subtiles per chunk
    NCHUNK = t // tc_tokens
    NBLK = nsh // 512      # pass-A score blocks
    NT = nsh // 128        # pass-B n-tiles
    NSTAT = NCHUNK * TSUB

    nc = bacc.Bacc("TRN2", target_bir_lowering=False, debug=False)
    # xT/eT are declared float32r: raw f32 bits from the host, consumed by the
    # tensor engine in its fast fp32 mode (internal mantissa truncation).
    # Avoids on-chip staging + rounding passes entirely.
    xT_d = nc.dram_tensor("xT", [d, t], F32R, kind="ExternalInput").ap()
    eT_d = nc.dram_tensor("eT", [d, nsh], F32R, kind="ExternalInput").ap()
    e_d = nc.dram_tensor("e", [nsh, d], BF16, kind="ExternalInput").ap()
    o_d = nc.dram_tensor("o", [t, d], F32, kind="ExternalOutput").ap()
    m_d = nc.dram_tensor("m", [128, NSTAT], F32, kind="ExternalOutput").ap()
    l_d = nc.dram_tensor("l", [128, NSTAT], F32, kind="ExternalOutput").ap()

    with tile.TileContext(nc) as tc:
        with (
            tc.tile_pool(name="pers", bufs=1) as pers,
            tc.tile_pool(name="scr", bufs=2) as scr,
            tc.tile_pool(name="pxt", bufs=2) as pxt,
            tc.tile_pool(name="pe", bufs=2) as pe_,
            tc.tile_pool(name="ppt", bufs=2) as ppt,
            tc.tile_pool(name="pout", bufs=2) as pout,
            tc.tile_pool(name="stt", bufs=2) as stt,
            tc.tile_pool(name="psA", bufs=2, space="PSUM") as psA,
            tc.tile_pool(name="psT", bufs=2, space="PSUM") as psT,
            tc.tile_pool(name="psAcc", bufs=1, space="PSUM") as psAcc,
        ):
            # --- persistent tiles ---
            et_r = pers.tile([128, KC, nsh], F32R, tag="etr")
            ident = pers.tile([128, 128], BF16, tag="id")
            m_all = pers.tile([128, NSTAT], F32, tag="mall")
            l_all = pers.tile([128, NSTAT], F32, tag="lall")
            make_identity(nc, ident)

            xT_r3 = xT_d.rearrange("(kc p) t -> p kc t", p=128)
            e_r3 = e_d.rearrange("(nt p) d -> p nt d", p=128)

            # chunk-0 xT first (so mm1 isn't queued behind the full eT load),
            # then resident E^T via direct f32r DMA, n-window-major so the
            # first mm1 blocks unblock after ~4 MiB instead of the whole 16 MiB
            xt0 = pxt.tile([128, KC, tc_tokens], F32R, tag="xt", name="xt0")
            nc.sync.dma_start(xt0[:], xT_r3[:, :, 0:tc_tokens])

            eT_r3 = eT_d.rearrange("(kc p) n -> p kc n", p=128)
            NWIN = max(1, nsh // 1024)
            WIN = nsh // NWIN
            for w in range(NWIN):
                for k in range(KC):
                    nc.sync.dma_start(
                        et_r[:, k, w * WIN:(w + 1) * WIN],
                        eT_r3[:, k, w * WIN:(w + 1) * WIN],
                    )

            for c in range(NCHUNK):
                # xT chunk: direct f32r DMA
                if c == 0:
                    xt = xt0
                else:
                    xt = pxt.tile([128, KC, tc_tokens], F32R, tag="xt",
                                  name=f"xt{c}")
                    nc.sync.dma_start(
                        xt[:], xT_r3[:, :, c * tc_tokens:(c + 1) * tc_tokens]
                    )

                scores = [scr.tile([128, nsh], F32, tag="scores", name=f"scores{c}_{s}") for s in range(TSUB)]
                mparts = stt.tile([128, TSUB, NBLK], F32, tag="mparts")
                negm = stt.tile([128, TSUB], F32, tag="negm")
                lparts = stt.tile([128, TSUB, NBLK], F32, tag="lparts")
                lsum = stt.tile([128, TSUB], F32, tag="lsum")
                linv = stt.tile([128, TSUB], F32, tag="linv")

                # ---- pass A: scores + row max ----
                for s in range(TSUB):
                    for j in range(NBLK):
                        ps = psA.tile([128, 512], F32, tag="mm1", name=f"psA{c}_{s}_{j}")
                        for k in range(KC):
                            nc.tensor.matmul(
                                ps[:],
                                xt[:, k, s * 128:(s + 1) * 128],
                                et_r[:, k, j * 512:(j + 1) * 512],
                                start=(k == 0),
                                stop=(k == KC - 1),
                            )
                        nc.vector.reduce_max(mparts[:, s, j:j + 1], ps[:], axis=AX)
                        nc.vector.tensor_copy(scores[s][:, j * 512:(j + 1) * 512], ps[:])
                    nc.vector.reduce_max(
                        negm[:, s:s + 1], mparts[:, s, :], axis=AX, negate=True
                    )

                # ---- pass B: P = exp(s - m), P^T, acc += P^T.T @ E ----
                acc = [psAcc.tile([128, d], F32, tag=f"acc{s}", name=f"acc{c}_{s}") for s in range(TSUB)]

                # software-pipelined at j-block granularity: iteration j does
                # [exp(j) on ACT] [all 8 transposes of block j -> one PSUM
                # bank] [one DVE copy -> SBUF] then the 16 mm2 matmuls of
                # block j-1 (whose P^T landed during block j's transposes).
                # Keeps the PE FIFO free of not-yet-ready work.
                pending = None

                def emit_mm2(pend):
                    ptq_sbp, e4p, jp = pend
                    for ii in range(4):
                        i = jp * 4 + ii
                        for s in range(TSUB):
                            for dh in range(d // 512):
                                nc.tensor.matmul(
                                    acc[s][:, dh * 512:(dh + 1) * 512],
                                    ptq_sbp[:, ii, s * 128:(s + 1) * 128],
                                    e4p[:, ii, dh * 512:(dh + 1) * 512],
                                    start=(i == 0),
                                    stop=(i == NT - 1),
                                )

                # P = exp(scores - m) is written IN PLACE into the low half of
                # the scores tile (bf16 view): block j's output lands in bytes
                # whose f32 scores were already consumed by block <= j/2, so the
                # next chunk's score copies only WAR against early-pass-B work.
                pviews = [scores[s].bitcast(BF16) for s in range(TSUB)]
                for j in range(NBLK):
                    pts = []
                    for s in range(TSUB):
                        p_t = pviews[s][:, j * 512:(j + 1) * 512]
                        nc.scalar.activation(
                            p_t,
                            scores[s][:, j * 512:(j + 1) * 512],
                            EXP,
                            bias=negm[:, s:s + 1],
                            scale=1.0,
                            accum_out=lparts[:, s, j:j + 1],
                        )
                        pts.append(p_t)
                    e4 = pe_.tile([128, 4, d], BF16, tag="e", name=f"e{c}_{j}")
                    nc.sync.dma_start(e4[:], e_r3[:, j * 4:(j + 1) * 4, :])
                    ptq_sb = ppt.tile([128, 4, TSUB * 128], BF16, tag="ptsb",
                                      name=f"ptqsb{c}_{j}")
                    # two psum tiles (distinct banks) so the copy of half 0 can
                    # run while half 1's transposes still write their own bank
                    # (same-bank PE-write + DVE-read is a hardware fault)
                    for hh in range(2):
                        ptq = psT.tile([128, 2, TSUB * 128], BF16, tag="ptps",
                                       name=f"ptq{c}_{j}_{hh}")
                        for i2 in range(2):
                            ii = hh * 2 + i2
                            for s in range(TSUB):
                                nc.tensor.transpose(
                                    ptq[:, i2, s * 128:(s + 1) * 128],
                                    pts[s][:, ii * 128:(ii + 1) * 128],
                                    ident[:],
                                )
                        nc.vector.tensor_copy(
                            ptq_sb[:, hh * 2:hh * 2 + 2], ptq[:]
                        )
                    del pts
                    if pending is not None:
                        emit_mm2(pending)
                    pending = (ptq_sb, e4, j)
                emit_mm2(pending)

                # ---- finalize chunk: normalize + store ----
                for s in range(TSUB):
                    sidx = c * TSUB + s
                    nc.vector.reduce_sum(lsum[:, s:s + 1], lparts[:, s, :], axis=AX)
                    nc.vector.reciprocal(linv[:, s:s + 1], lsum[:, s:s + 1])
                    o_t = pout.tile([128, d], F32, tag="ot")
                    nc.vector.tensor_scalar_mul(o_t[:], acc[s][:], linv[:, s:s + 1])
                    t0 = c * tc_tokens + s * 128
                    nc.sync.dma_start(o_d[t0:t0 + 128, :], o_t[:])
                    nc.vector.tensor_scalar_mul(
                        m_all[:, sidx:sidx + 1], negm[:, s:s + 1], -1.0
                    )
                    nc.vector.tensor_copy(l_all[:, sidx:sidx + 1], lsum[:, s:s + 1])

            nc.sync.dma_start(m_d[:], m_all[:])
            nc.sync.dma_start(l_d[:], l_all[:])

    if do_compile:
        nc.compile()
    return nc


_NC_CACHE = {}


def _get_nc():
    if "nc" not in _NC_CACHE:
        _NC_CACHE["nc"] = build_nc()
    return _NC_CACHE["nc"]


def kernel(x, embeddings):
    out, _ = run_hw(x, embeddings)
    return out


def run_hw(x, embeddings, **spmd_kwargs):
    x = np.asarray(x, dtype=np.float32)
    embeddings = np.asarray(embeddings, dtype=np.float32)
    assert x.shape == (T, D) and embeddings.shape == (N, D)

    nc = _get_nc()

    xT = np.ascontiguousarray(x.T)
    ET = embeddings.T
    in_maps = []
    for c in range(NCORES):
        sl = slice(c * NSH, (c + 1) * NSH)
        in_maps.append(
            {
                "xT": xT,
                "eT": np.ascontiguousarray(ET[:, sl]),
                "e": embeddings[sl].astype(ml_dtypes.bfloat16),
            }
        )

    res = run_bass_kernel_spmd(nc, in_maps, list(range(NCORES)), **spmd_kwargs)
    return combine(res.results), res


def combine(results):
    """Host-side softmax combine across the 8 N-shards."""
    o = np.stack([r["o"] for r in results])  # [C, T, D] f32, each normalized by l_c
    # m/l tiles are [128 partitions, T/128 subtiles]; token t = sidx*128 + p
    m = np.stack([r["m"].T.reshape(-1) for r in results]).astype(np.float64)  # [C, T]
    l = np.stack([r["l"].T.reshape(-1) for r in results]).astype(np.float64)  # [C, T]
    M = m.max(axis=0)
    w = l * np.exp(m - M)
    w /= w.sum(axis=0)
    out = np.einsum("ct,ctd->td", w, o.astype(np.float64))
    return out.astype(np.float32)

